# revision 2
# baseline (speedup 1.0000x reference)
"""MoE routing kernel for TRN2, SPMD over 8 NeuronCores.

Problem (per reference):
  x = mean(hidden_states, axis=1)                  # [B, H]
  scores = x @ gate_w + gate_b                     # [B, E]
  weights, sel = top_k(scores, 2)
  all_out = einsum('bh,eho->beo', x, expert_w) + expert_b
  out = sum(weights * all_out[b, sel], axis=1)     # [B, H]

Shapes: B=2048, S=256, H=1024, E=8, TOPK=2, fp32.

Design: the 256 MiB/core hidden_states stream is the bottleneck, and a
single DMA queue on this platform sustains only ~90-180 GB/s, so the
mean-over-S reduction is split across all three DMA paths concurrently:
  - Pool/SWDGE queue: accumulate chains (dma_start accum_op=add) -- the
    DMA engines' inline CCE adders do the reduction in-flight;
  - the two HWDGE queues (sync/qSP, scalar/qAct): plain 1 MB streaming
    loads into staging tiles, folded by DVE tensor_adds.
Token-tile 0 is processed fully before token-tile 1 so its merge /
transpose / gate overlaps tile 1's DMA stream. Expert weights stream on
qSP as f32r (same bits as fp32; DRAM tensor declared f32r so HWDGE needs
no cast) and overlap the phase-1 stream; expert matmuls run f32r at full
PE rate (N=512).

Measured (reps=128 device-resident timing): ~1.20 ms/core vs 2.13 ms for
the previous DVE-tensor_reduce version on the same methodology (~1.76x).
"""

import numpy as np

B, S, H, E = 2048, 256, 1024, 8
N_CORES = 8
B_LOC = B // N_CORES          # 256 tokens per core
N_TT = B_LOC // 128           # 2 token-tiles of 128
KC = H // 128                 # 8 contraction chunks
NCH = H // 512                # 2 output column chunks
NCHAIN = 4                    # accum chains per token-tile
G = 2                         # consecutive s-values per link (1 MB DMAs)
NPAIR = S // G                # 128 s-pairs per token-tile
N_ALT = 64                    # s-pairs per tile on the plain-DMA+DVE path
NSTG = 4                      # staging buffers for the plain path
NPACC = NPAIR - N_ALT         # pairs handled by accum chains
LINKS = NPACC // NCHAIN       # 16 links per chain

_compiled = {}


def _build(reps=1):
    import concourse.bacc as bacc
    import concourse.mybir as mybir
    import concourse.tile as tile
    from concourse.masks import make_identity

    fp32 = mybir.dt.float32
    f32r = mybir.dt.float32r
    nc = bacc.Bacc("TRN2", target_bir_lowering=False, debug=False,
                   num_devices=N_CORES)

    hs = nc.dram_tensor("hidden_states", [B_LOC, S, H], fp32,
                        kind="ExternalInput").ap()
    gate_w = nc.dram_tensor("gate_w", [H, E], fp32, kind="ExternalInput").ap()
    gate_b = nc.dram_tensor("gate_b", [E], fp32, kind="ExternalInput").ap()
    # f32r has identical bits to fp32; declaring the DRAM tensor f32r lets
    # HWDGE (sync) load the expert weights with no cast.
    expert_w = nc.dram_tensor("expert_w", [E, H, H], f32r,
                              kind="ExternalInput").ap()
    expert_b = nc.dram_tensor("expert_b", [E, H], fp32,
                              kind="ExternalInput").ap()
    out = nc.dram_tensor("out", [B_LOC, H], fp32, kind="ExternalOutput").ap()

    with tile.TileContext(nc) as tc:
        with (
            tc.tile_pool(name="chain", bufs=1) as chain_pool,
            tc.tile_pool(name="w", bufs=12) as w_pool,
            tc.tile_pool(name="acc", bufs=1) as acc_pool,
            tc.tile_pool(name="small", bufs=1) as small_pool,
            tc.tile_pool(name="top2", bufs=1) as top2_pool,
            tc.tile_pool(name="psum", bufs=4, space="PSUM") as psum_pool,
            tc.tile_pool(name="psmall", bufs=2, space="PSUM") as psmall_pool,
        ):
            # --- constants / small inputs (outside the rep loop) ---
            identity = small_pool.tile([128, 128], fp32, tag="ident")
            make_identity(nc, identity[:])
            ones_row = small_pool.tile([1, 128], fp32, tag="ones")
            nc.vector.memset(ones_row[:], 1.0)

            gw_s = small_pool.tile([128, KC * E], fp32, tag="gw")
            for kc in range(KC):
                nc.sync.dma_start(
                    out=gw_s[:, kc * E:(kc + 1) * E],
                    in_=gate_w[kc * 128:(kc + 1) * 128, :])
            gb_s = small_pool.tile([1, E], fp32, tag="gb")
            nc.sync.dma_start(out=gb_s[:], in_=gate_b[None, :])
            eb_s = small_pool.tile([E, H], fp32, tag="eb")
            nc.sync.dma_start(out=eb_s[:], in_=expert_b[:, :])

            def body():
                # --- phase 1: mean over S, token-tile 0 fully first so its
                # merge/transpose/gate overlaps token-tile 1's DMA stream.
                # Per tile: NCHAIN accum chains (Pool SWDGE) cover pairs
                # p = l*NCHAIN+j < NPACC; N_ALT trailing pairs stream via
                # the second HWDGE queue (scalar/qAct) + DVE adds.
                xT = []
                xTr = []
                for kc in range(KC):
                    xT.append(acc_pool.tile([128, B_LOC], fp32,
                                            tag=f"xt{kc}", name=f"xt{kc}"))
                    xTr.append(acc_pool.tile([128, B_LOC], f32r,
                                             tag=f"xtr{kc}", name=f"xtr{kc}"))
                xsums = []
                for tt in range(N_TT):
                    a = [chain_pool.tile([128, G * H], fp32,
                                         tag=f"c{tt}_{j}", name=f"c{tt}_{j}")
                         for j in range(NCHAIN)]
                    # init links (HWDGE/sync)
                    for j in range(NCHAIN):
                        s0 = G * j
                        nc.sync.dma_start(
                            out=a[j][:].rearrange("p (s h) -> p s h", s=G),
                            in_=hs[tt * 128:(tt + 1) * 128, s0:s0 + G, :])
                    # plain path: N_ALT pairs stream via the two HWDGE
                    # queues (sync/qSP, scalar/qAct) into staging tiles;
                    # DVE folds them into dacc. Interleaved with the accum
                    # links below by the Tile scheduler.
                    dacc = chain_pool.tile([128, H], fp32, tag=f"da{tt}",
                                           name=f"da{tt}")
                    stgs = []
                    for k in range(N_ALT):
                        s0 = G * (NPACC + k)
                        stg = chain_pool.tile([128, G * H], fp32,
                                              tag=f"stg{k % NSTG}",
                                              name=f"stg{k}")
                        eng = nc.scalar if (k % 2 == 0) else nc.sync
                        eng.dma_start(
                            out=stg[:].rearrange("p (s h) -> p s h", s=G),
                            in_=hs[tt * 128:(tt + 1) * 128, s0:s0 + G, :])
                        stgs.append(stg)
                    # accum links (Pool SWDGE)
                    for l in range(1, LINKS):
                        for j in range(NCHAIN):
                            p = l * NCHAIN + j
                            s0 = G * p
                            nc.gpsimd.dma_start(
                                out=a[j][:].rearrange("p (s h) -> p s h",
                                                      s=G),
                                in_=hs[tt * 128:(tt + 1) * 128,
                                       s0:s0 + G, :],
                                accum_op=mybir.AluOpType.add)
                    # DVE: fold plain stages into dacc as they land
                    for k in range(N_ALT):
                        if k == 0:
                            nc.vector.tensor_add(dacc[:], stgs[0][:, :H],
                                                 stgs[0][:, H:])
                        else:
                            nc.vector.tensor_add(dacc[:], dacc[:],
                                                 stgs[k][:, :H])
                            nc.vector.tensor_add(dacc[:], dacc[:],
                                                 stgs[k][:, H:])
                    # fold chains + alt accumulator down to x-sum, scale
                    for j in range(NCHAIN):
                        nc.vector.tensor_add(a[j][:, :H], a[j][:, :H],
                                             a[j][:, H:])
                    nc.vector.tensor_add(a[0][:, :H], a[0][:, :H], a[1][:, :H])
                    nc.vector.tensor_add(a[2][:, :H], a[2][:, :H], a[3][:, :H])
                    nc.vector.tensor_add(a[0][:, :H], a[0][:, :H], a[2][:, :H])
                    nc.vector.tensor_add(a[0][:, :H], a[0][:, :H], dacc[:])
                    # x = sum / S (PE transpose ignores identity values, so
                    # the 1/S scale must be a real DVE op)
                    nc.vector.tensor_scalar_mul(a[0][:, :H], a[0][:, :H],
                                                1.0 / S)
                    xsums.append(a[0])
                    # transpose this tile's x into xT/xTr columns
                    for kc in range(KC):
                        pt = psmall_pool.tile([128, 128], fp32, tag="pt")
                        nc.tensor.transpose(
                            pt[:], a[0][:, kc * 128:(kc + 1) * 128],
                            identity[:])
                        nc.vector.tensor_copy(
                            out=xT[kc][:, tt * 128:(tt + 1) * 128], in_=pt[:])
                        nc.vector.tensor_copy(
                            out=xTr[kc][:, tt * 128:(tt + 1) * 128],
                            in_=pt[:])

                # --- phase 3: gate scores + top-2 mask weights ---
                m_tiles = []   # [128, E] combine weights per token-tile
                mT_tiles = []  # [E, 128] transposed
                for tt in range(N_TT):
                    ps_sc = psmall_pool.tile([128, E], fp32, tag="pt")
                    for kc in range(KC):
                        nc.tensor.matmul(
                            ps_sc[:], xT[kc][:, tt * 128:(tt + 1) * 128],
                            gw_s[:, kc * E:(kc + 1) * E],
                            start=(kc == 0), stop=False)
                    nc.tensor.matmul(ps_sc[:], ones_row[:], gb_s[:],
                                     start=False, stop=True)
                    s_t = top2_pool.tile([128, E], fp32, tag=f"s{tt}")
                    nc.vector.tensor_copy(out=s_t[:], in_=ps_sc[:])
                    max1 = top2_pool.tile([128, 1], fp32, tag=f"mx1{tt}")
                    nc.vector.tensor_reduce(
                        max1[:], s_t[:], mybir.AxisListType.X,
                        mybir.AluOpType.max)
                    ge1 = top2_pool.tile([128, E], fp32, tag=f"ge1{tt}")
                    nc.vector.tensor_scalar(
                        ge1[:], s_t[:], max1[:], None, mybir.AluOpType.is_ge)
                    masked = top2_pool.tile([128, E], fp32, tag=f"msk{tt}")
                    nc.vector.scalar_tensor_tensor(
                        out=masked[:], in0=ge1[:], scalar=-1e30, in1=s_t[:],
                        op0=mybir.AluOpType.mult, op1=mybir.AluOpType.add)
                    max2 = top2_pool.tile([128, 1], fp32, tag=f"mx2{tt}")
                    nc.vector.tensor_reduce(
                        max2[:], masked[:], mybir.AxisListType.X,
                        mybir.AluOpType.max)
                    ge2 = top2_pool.tile([128, E], fp32, tag=f"ge2{tt}")
                    nc.vector.tensor_scalar(
                        ge2[:], s_t[:], max2[:], None, mybir.AluOpType.is_ge)
                    m_t = top2_pool.tile([128, E], fp32, tag=f"m{tt}")
                    nc.vector.tensor_mul(m_t[:], s_t[:], ge2[:])
                    m_tiles.append(m_t)
                    # transpose m -> mT [E, 128] (unscaled identity!)
                    pmT = psmall_pool.tile([E, 128], fp32, tag="pt")
                    nc.tensor.transpose(pmT[:], m_t[:], identity[:])
                    mT = top2_pool.tile([E, 128], fp32, tag=f"mT{tt}")
                    nc.vector.tensor_copy(out=mT[:], in_=pmT[:])
                    mT_tiles.append(mT)

                # --- phase 4: init out_acc with combined bias m @ expert_b ---
                out_accs = []
                for tt in range(N_TT):
                    oa = acc_pool.tile([128, H], fp32, tag=f"oa{tt}")
                    for nch in range(NCH):
                        pb = psum_pool.tile([128, 512], fp32, tag="ps")
                        nc.tensor.matmul(
                            pb[:], mT_tiles[tt][:],
                            eb_s[:, nch * 512:(nch + 1) * 512],
                            start=True, stop=True)
                        nc.vector.tensor_copy(
                            out=oa[:, nch * 512:(nch + 1) * 512], in_=pb[:])
                    out_accs.append(oa)

                # --- phase 5: experts ---
                for e in range(E):
                    w_tiles = []
                    for kc in range(KC):
                        wt = w_pool.tile([128, H], f32r, tag="w")
                        nc.sync.dma_start(
                            out=wt[:],
                            in_=expert_w[e, kc * 128:(kc + 1) * 128, :])
                        w_tiles.append(wt)
                    for tt in range(N_TT):
                        for nch in range(NCH):
                            ps = psum_pool.tile([128, 512], fp32, tag="ps")
                            for kc in range(KC):
                                nc.tensor.matmul(
                                    ps[:],
                                    xTr[kc][:, tt * 128:(tt + 1) * 128],
                                    w_tiles[kc][:, nch * 512:(nch + 1) * 512],
                                    start=(kc == 0), stop=(kc == KC - 1))
                            sl = out_accs[tt][:, nch * 512:(nch + 1) * 512]
                            nc.vector.scalar_tensor_tensor(
                                out=sl, in0=ps[:],
                                scalar=m_tiles[tt][:, e:e + 1],
                                in1=sl, op0=mybir.AluOpType.mult,
                                op1=mybir.AluOpType.add)

                # --- phase 6: store ---
                for tt in range(N_TT):
                    nc.sync.dma_start(out=out[tt * 128:(tt + 1) * 128, :],
                                      in_=out_accs[tt][:])

            if reps == 1:
                body()
            else:
                with tc.For_i(0, reps, 1):
                    body()

    nc.compile()
    return nc


def _get_compiled(reps=1):
    if reps not in _compiled:
        _compiled[reps] = _build(reps)
    return _compiled[reps]


def kernel(**inputs):
    from concourse.bass_utils import run_bass_kernel_spmd

    reps = int(inputs.pop("_reps", 1))
    hs = np.ascontiguousarray(np.asarray(inputs["hidden_states"],
                                         dtype=np.float32))
    gw = np.ascontiguousarray(np.asarray(inputs["gate_w"], dtype=np.float32))
    gb = np.ascontiguousarray(np.asarray(inputs["gate_b"], dtype=np.float32))
    ew = np.ascontiguousarray(np.asarray(inputs["expert_w"],
                                         dtype=np.float32))
    eb = np.ascontiguousarray(np.asarray(inputs["expert_b"],
                                         dtype=np.float32))

    nc = _get_compiled(reps)
    in_maps = []
    for i in range(N_CORES):
        in_maps.append({
            "hidden_states": hs[i * B_LOC:(i + 1) * B_LOC],
            "gate_w": gw,
            "gate_b": gb,
            "expert_w": ew,
            "expert_b": eb,
        })
    res = run_bass_kernel_spmd(nc, in_maps, list(range(N_CORES)), trace=False)
    return np.concatenate([res.results[i]["out"] for i in range(N_CORES)],
                          axis=0)
